# revision 4
# baseline (speedup 1.0000x reference)
"""Expert-parallel grouped GEMM (MoE) kernel for Trainium2.

Problem: inputs [65536, 1024] sorted by expert (8192 tokens/expert),
weight [8, 512, 1024]; out[t] = x[t] @ W[expert(t)].T -> [65536, 512].

Sharding: expert-parallel across 8 NeuronCores. Tokens are already sorted
by expert and expert_size is static, so core e simply takes token rows
[e*8192:(e+1)*8192] and weight[e] - no all-to-all needed.

Device kernel (per core): one [8192,1024] @ [1024,512] GEMM.

v2: hybrid fp16 + fp8-DoubleRow split-K. The PE streams one moving
column per cycle regardless of dtype, so the fp16 kernel is pinned at
~109us (262144 col-cycles @ 2.4GHz). DoubleRow packs 2 fp8 weights per
PE cell (contraction 256 per matmul) for ~2x MACs/cycle, but pure e4m3
costs 3.8e-2 rel err (> 2e-2 gate). Hybrid: contraction k=0..767 in
fp16 (6 k-tiles), k=768..1023 as ONE DoubleRow fp8 matmul (2 packed
k-tiles), both accumulating into the same PSUM bank. Predicted error
1.88e-2 (numpy sim of e4m3 RNE + fp32 accum), PE time ~0.89x.

- Scale trick: w rows ~ N(0, 1/1024) land in e4m3 subnormals, so both
  w16 and w8 are pre-scaled by 32 host-side (exact power of two; w*32 ~
  N(0,1) is centered for e4m3). PSUM then holds 32*out and the final
  PSUM->SBUF copy becomes tensor_scalar_mul(1/32) - same DVE cost as
  the tensor_copy it replaces.
- Both DoubleRow operands are 3D APs [128, 2, F]: contraction index =
  j*128 + p for subtile j, partition p - i.e. two adjacent k-tiles in
  the same k-major SBUF layout the fp16 tiles already use.
- x stationary per token-tile ([128,2,128] for fp8, LDWEIGHTS 256 cols,
  no FWL); w moving ([128,2,512] fp8 / [128,512] fp16). Weight tiles
  stay resident in SBUF; x streams in prefetched blocks.
"""

import numpy as np

E = 8          # experts == cores
O = 512        # out_features
I = 1024       # in_features
S = 8192       # tokens per expert
K16T = 6       # fp16 k-tiles (contraction 0..767)
K16 = K16T * 128
K8T = 2        # fp8 k-tiles packed into one DoubleRow matmul (768..1023)
K8 = K8T * 128
WSCALE = 32.0  # power-of-2 weight pre-scale (undone in the PSUM copy)
S_BLK = 2048   # max tokens per streamed x block
BLOCKS = (512, 1536, 2048, 2048, 1536, 512)  # ramp up AND down, sums to S
X_BUFS = 4     # x block buffers (prefetch depth)
FP8 = True     # hybrid split-K; False reproduces the all-fp16 baseline
OUT_B = 4      # t-tiles batched per output DMA

assert K16 + (K8 if FP8 else 0) == I or not FP8
if not FP8:
    K16T, K16 = 8, 1024  # plain fp16 over the full contraction

_cache = {}


def _build_nc(repeats=1, loop=0, idle=0):
    import concourse.bass as bass
    import concourse.tile as tile
    from concourse import bacc, mybir
    from contextlib import nullcontext

    in_dt = mybir.dt.float16
    f8_dt = mybir.dt.float8e4
    blocks = []  # (start_token, n_tokens)
    pos = 0
    for sz in BLOCKS:
        blocks.append((pos, sz))
        pos += sz
    assert pos == S and all(sz % 128 == 0 and sz <= S_BLK for _, sz in blocks)

    nc = bacc.Bacc("TRN2", target_bir_lowering=False, debug=False)
    xT = nc.dram_tensor("xT", [K16, S], in_dt, kind="ExternalInput")
    wT = nc.dram_tensor("wT", [K16, O], in_dt, kind="ExternalInput")
    if FP8:
        x8T = nc.dram_tensor("x8T", [K8, S], f8_dt, kind="ExternalInput")
        w8T = nc.dram_tensor("w8T", [K8, O], f8_dt, kind="ExternalInput")
    outT = nc.dram_tensor("out", [S, O], mybir.dt.float16, kind="ExternalOutput")
    if idle:
        ping = nc.dram_tensor("ping", [1, 8], mybir.dt.float16)
        pong = nc.dram_tensor("pong", [1, 8], mybir.dt.float16)

    with tile.TileContext(nc) as tc:
        with (
            tc.tile_pool(name="wpool", bufs=1) as wpool,
            tc.tile_pool(name="xpool", bufs=X_BUFS) as xpool,
            tc.tile_pool(name="opool", bufs=4) as opool,
            tc.tile_pool(name="psum", bufs=8, space=bass.MemorySpace.PSUM) as psum_pool,
        ):
            wt = wpool.tile([128, K16T * O], in_dt)
            w8t = (wpool.tile([128, K8T, O], f8_dt, name="w8t")
                   if FP8 else None)

            def load_block(blk, with_weights=False):
                # with_weights: interleave the resident-weight k-tile loads
                # with this block's stripes so the first matmul (needs only
                # wt[k=0] + stripe[k=0]) starts ~5us earlier than with a
                # serial full-weight prefix.
                s0, sz = blk
                xblk = xpool.tile([128, K16T * sz], in_dt, tag="xblk")
                x8blk = (xpool.tile([128, K8T, sz], f8_dt, tag="x8blk",
                                    name="x8blk")
                         if FP8 else None)
                for k in range(K16T):
                    if with_weights:
                        nc.sync.dma_start(wt[:, k * O:(k + 1) * O],
                                          wT[k * 128:(k + 1) * 128, :])
                    nc.sync.dma_start(
                        xblk[:, k * sz:(k + 1) * sz],
                        xT[k * 128:(k + 1) * 128, s0:s0 + sz],
                    )
                if FP8:
                    for j in range(K8T):
                        if with_weights:
                            nc.sync.dma_start(w8t[:, j, :],
                                              w8T[j * 128:(j + 1) * 128, :])
                        nc.sync.dma_start(
                            x8blk[:, j, :],
                            x8T[j * 128:(j + 1) * 128, s0:s0 + sz],
                        )
                return xblk, x8blk

            last_ot = [None]

            def compute_block(blk, xblk, x8blk):
                s0, sz = blk
                for tg in range(sz // 128 // OUT_B):
                    ot = opool.tile([128, OUT_B, O], mybir.dt.float16, tag="ot")
                    for ti in range(OUT_B):
                        t = tg * OUT_B + ti
                        ps = psum_pool.tile([128, O], mybir.dt.float32,
                                            name="ps", tag="ps")
                        for k in range(K16T):
                            nc.tensor.matmul(
                                ps[:],
                                xblk[:, k * sz + t * 128: k * sz + (t + 1) * 128],
                                wt[:, k * O:(k + 1) * O],
                                start=(k == 0),
                                stop=(k == K16T - 1) and not FP8,
                            )
                        if FP8:
                            nc.tensor.matmul(
                                ps[:],
                                x8blk[:, :, t * 128:(t + 1) * 128],
                                w8t[:],
                                start=False,
                                stop=True,
                                perf_mode=mybir.MatmulPerfMode.DoubleRow,
                            )
                        nc.vector.tensor_scalar_mul(ot[:, ti, :], ps[:],
                                                    1.0 / WSCALE)
                    g0 = s0 + tg * OUT_B * 128
                    dst = outT[g0:g0 + OUT_B * 128, :].rearrange(
                        "(t p) o -> p t o", p=128)
                    nc.scalar.dma_start(dst, ot[:])
                    last_ot[0] = ot[:, 0, :]

            loop_cm = (
                tc.For_i(0, loop, 1,
                         hint_engines=(mybir.EngineType.PE, mybir.EngineType.SP,
                                       mybir.EngineType.DVE))
                if loop else nullcontext()
            )
            with loop_cm:
                for _ in range(repeats):
                    pending = []  # (blk, xblk, x8blk) loaded but not computed
                    for bi, blk in enumerate(blocks):
                        pending.append((blk, *load_block(blk, with_weights=bi == 0)))
                        if len(pending) >= X_BUFS:
                            compute_block(*pending.pop(0))
                    for args in pending:
                        compute_block(*args)
                # low-power idle: dependent tiny DMA ping-pong through one
                # SBUF tile (Tile tracks the tile's RAW/WAR deps, so the
                # copies serialize on each other's completion latency).
                # The first copy reads the gemm's final output tile, so the
                # idle runs strictly AFTER the gemm instead of alongside it,
                # and the per-iteration span is gemm_span + idle_span.
                # Keeps average chip power low so duty-cycled benchmarks see
                # the unthrottled PE clock.
                if idle:
                    idle_t = wpool.tile([1, 8], mybir.dt.float16, name="idle_t")
                    if last_ot[0] is not None:
                        nc.sync.dma_start(idle_t[:], last_ot[0][0:1, 0:8])
                    for i in range(idle):
                        if i % 2 == 0:
                            nc.sync.dma_start(pong[:], idle_t[:])
                        else:
                            nc.sync.dma_start(idle_t[:], ping[:])
    nc.compile()
    return nc


def _get_nc(repeats=1, loop=0, idle=0):
    key = (repeats, loop, idle, BLOCKS, X_BUFS, FP8, K16T, OUT_B)
    if key not in _cache:
        _cache[key] = _build_nc(repeats, loop, idle)
    return _cache[key]


def make_in_maps(inputs, weight):
    """Per-core input tensors: transposed, scaled, split fp16/fp8."""
    import ml_dtypes

    f8 = ml_dtypes.float8_e4m3
    in_maps = []
    for e in range(E):
        x_e = inputs[e * S:(e + 1) * S, :]        # [S, I] fp32
        w_e = weight[e] * WSCALE                  # [O, I] fp32, pre-scaled
        m = {
            "xT": np.ascontiguousarray(x_e[:, :K16].T.astype(np.float16)),
            "wT": np.ascontiguousarray(w_e[:, :K16].T.astype(np.float16)),
        }
        if FP8:
            m["x8T"] = np.ascontiguousarray(x_e[:, K16:].T.astype(f8))
            m["w8T"] = np.ascontiguousarray(w_e[:, K16:].T.astype(f8))
        in_maps.append(m)
    return in_maps


def run(inputs, weight, trace=False, repeats=1, loop=0):
    """Shard, run on 8 cores, gather. Returns (out, BassKernelResults)."""
    from concourse.bass_utils import run_bass_kernel_spmd

    nc = _get_nc(repeats, loop)
    in_maps = make_in_maps(inputs, weight)
    res = run_bass_kernel_spmd(nc, in_maps, list(range(E)), trace=trace)
    outs = [res.results[e]["out"] for e in range(E)]
    out = np.concatenate([o.astype(np.float32) for o in outs], axis=0)
    return out, res


def kernel(inputs, weight, expert_size):
    inputs = np.asarray(inputs, dtype=np.float32)
    weight = np.asarray(weight, dtype=np.float32)
    assert inputs.shape == (E * S, I) and weight.shape == (E, O, I)
    assert int(expert_size) == S
    out, _ = run(inputs, weight, trace=False)
    return out
